# revision 19
# baseline (speedup 1.0000x reference)
"""BERT-style MLM forward on 8 TRN2 NeuronCores.

Data-parallel over batch (B=8 -> 1 sequence per core, no collectives).

v2 design:
- Residual stream is TOKEN-major [t%128, t//128, dm]: LayerNorm runs entirely
  on the Vector engine (bn_stats/bn_aggr + per-partition tensor_scalar), no
  TensorE stats matmuls and no cross-partition broadcasts.
- Q/K/fc1 GEMMs are weights-stationary feature-major (outputs land feature-
  major for attention / fc2). V/AO/fc2/fc GEMMs are activation-stationary
  (lhsT = activation chunk, rhs = weight slab streamed at N=512), producing
  token-major outputs directly.
- Attention computes scores TRANSPOSED ([k, q] = K^T-stationary), so softmax
  normalization is a ones-matmul partition reduction and attn^T feeds the ctx
  matmul directly -- no per-head PE transposes.
- bf16 matmul operands, fp32 PSUM/statistics. One-hot matmuls for embedding
  and masked-position gather (one-hots built on host). Loss replicated on
  host from device logits.
"""
import sys

try:
    import concourse.bass as bass  # noqa: F401
except ImportError:
    sys.path.insert(0, "/opt/trn_rl_repo")

import ml_dtypes
import numpy as np

import concourse.bass as bass
import concourse.bass_utils as _bu
import concourse.mybir as mybir
from concourse import bacc, tile
from concourse.bass_utils import run_bass_kernel_spmd
from concourse.masks import make_identity



B, T, NM = 8, 256, 40
V, NE, DM, DK, DVh, H, DFF, L = 30, 1024, 1024, 64, 64, 16, 4096, 8
EPS = 1e-5
P = 128
CD = DM // P   # 8
CF = DFF // P  # 32
TQ = T // P    # 2
SCALE = float(1.0 / np.sqrt(DK).astype(np.float32))

bf16 = mybir.dt.bfloat16
f32 = mybir.dt.float32
BF = ml_dtypes.bfloat16
AF = mybir.ActivationFunctionType
OP = mybir.AluOpType
AX = mybir.AxisListType


def build_graph():
    nc = bacc.Bacc()

    def param(name, shape, dt=bf16):
        return nc.declare_dram_parameter(name, list(shape), dt, isOutput=False)

    # per-core tensors
    seq_oh = param("seq_oh", (V, T))            # one-hot tokens [v, t]
    pos_oh = param("pos_oh", (T, NM))           # one-hot masked positions
    maskc = param("maskc", (P, TQ), f32)        # 0/1 key mask, [k%128, k//128]
    # weights
    te_w = param("te_w", (V, NE))
    pos_e = param("pos_e", (T, NE))             # pos_embed[0], token-major
    fc_w = param("fc_w", (NE, DM))
    wq_w = param("wq_w", (L, DM, H * DK))
    wk_w = param("wk_w", (L, DM, H * DK))
    wv_w = param("wv_w", (L, DM, H * DVh))
    ao_w = param("ao_w", (L, H * DVh, DM))
    f1_w = param("f1_w", (L, DM, DFF))
    f2_w = param("f2_w", (L, DFF, DM))
    lin_w = param("lin_w", (DM, DM))
    head_w = param("head_w", (DM, V))
    # per-partition column layouts [P, nchunks] (feature-major eviction biases)
    wq_b = param("wq_b", (L, P, CD), f32)
    wk_b = param("wk_b", (L, P, CD), f32)
    wv_b = param("wv_b", (L, P, CD), f32)
    f1_b = param("f1_b", (L, P, CF), f32)
    lin_b = param("lin_b", (P, CD), f32)
    # replicated row layouts [P, DM] (token-major: per-feature on free axis)
    fc_b = param("fc_b", (P, DM))
    ln1_g = param("ln1_g", (L, P, DM))
    ln1_b = param("ln1_b", (L, P, DM))
    ao_b = param("ao_b", (L, P, DM))
    f2_b = param("f2_b", (L, P, DM))
    logits_out = nc.declare_dram_parameter("logitsT", [V, NM], f32, isOutput=True)

    with tile.TileContext(nc) as tc:
        with tc.tile_pool(name="cst", bufs=1) as cst, \
             tc.tile_pool(name="acts", bufs=1) as acts, \
             tc.tile_pool(name="wsl", bufs=36) as wslp, \
             tc.tile_pool(name="reps", bufs=6) as reps, \
             tc.tile_pool(name="sm", bufs=2) as sm, \
             tc.tile_pool(name="psA", bufs=4, space="PSUM") as psA, \
             tc.tile_pool(name="psB", bufs=1, space="PSUM") as psB:

            ident = cst.tile([P, P], bf16, tag="ident")
            make_identity(nc, ident)
            onesk = cst.tile([P, 1], bf16, tag="onesk")
            nc.vector.memset(onesk, 1.0)
            ones1 = cst.tile([1, P], bf16, tag="ones1")
            nc.vector.memset(ones1, 1.0)
            eps_t = cst.tile([P, 1], f32, tag="eps")
            nc.vector.memset(eps_t, EPS)
            mask_sb = cst.tile([P, TQ], f32, tag="mask")
            nc.sync.dma_start(out=mask_sb, in_=maskc[:, :])

            def ldvec(ap2d, nchunk):
                t = sm.tile([P, nchunk], f32, tag="vecs", name="vec")
                nc.sync.dma_start(out=t, in_=ap2d)
                return t

            def ldrep(ap2d):
                t = reps.tile([P, DM], bf16, tag="rep", name="rep")
                nc.sync.dma_start(out=t, in_=ap2d)
                return t

            # ---- token-major layernorm: all-DVE + tiny ACT sqrt ----
            def ln_tok(x, g_rep, b_rep, out):
                for tt in range(TQ):
                    stats = sm.tile([P, 2, 6], f32, tag="ln_st", name="ln_st")
                    for sg in range(2):
                        nc.vector.bn_stats(out=stats[:, sg, :], in_=x[:, tt, sg * 512:(sg + 1) * 512])
                    mv = sm.tile([P, 2], f32, tag="ln_mv", name="ln_mv")
                    nc.vector.bn_aggr(out=mv[:, :], in_=stats[:, :, :])
                    std = sm.tile([P, 1], f32, tag="ln_sd", name="ln_sd")
                    nc.scalar.activation(out=std[:, :], in_=mv[:, 1:2], func=AF.Sqrt,
                                         bias=eps_t[:, :], scale=1.0)
                    nc.vector.reciprocal(out=std[:, :], in_=std[:, :])
                    if g_rep is None:
                        nc.vector.tensor_scalar(out=out[:, tt, :], in0=x[:, tt, :],
                                                scalar1=mv[:, 0:1], scalar2=std[:, :],
                                                op0=OP.subtract, op1=OP.mult)
                    else:
                        tmp = sm.tile([P, DM], bf16, tag="ln_t", name="ln_t")
                        nc.vector.tensor_scalar(out=tmp[:, :], in0=x[:, tt, :],
                                                scalar1=mv[:, 0:1], scalar2=std[:, :],
                                                op0=OP.subtract, op1=OP.mult)
                        nc.vector.tensor_tensor(out=tmp[:, :], in0=tmp[:, :], in1=g_rep[:, :], op=OP.mult)
                        nc.vector.tensor_tensor(out=out[:, tt, :], in0=tmp[:, :], in1=b_rep[:, :], op=OP.add)

            # ---- token-major -> feature-major transpose ----
            def t2f(x_tok, out_feat):
                for cg in range(2):
                    tp = psB.tile([P, 8, P], bf16, tag="sc", name="tp")
                    for ci in range(4):
                        c = cg * 4 + ci
                        for tt in range(TQ):
                            nc.tensor.transpose(tp[:, ci * 2 + tt, :], x_tok[:, tt, c * P:(c + 1) * P],
                                                ident[:, :])
                    for ci in range(4):
                        c = cg * 4 + ci
                        nc.vector.tensor_copy(out_feat[:, c, :], tp[:, ci * 2:ci * 2 + 2, :])

            # ---- weights-stationary feature-major GEMM (m-outer, slabs resident) ----
            def gemm_ws(w_dram, act_in, cin, mout, evict):
                slabs = []
                for c in range(cin):
                    wsl = wslp.tile([P, mout * P], bf16, tag="wslab", name="wsl")
                    nc.sync.dma_start(out=wsl, in_=w_dram[c * P:(c + 1) * P, :])
                    slabs.append(wsl)
                for m in range(mout):
                    ps = psA.tile([P, T], f32, tag="gps", name="gps")
                    for c in range(cin):
                        nc.tensor.matmul(ps[:, :], lhsT=slabs[c][:, m * P:(m + 1) * P],
                                         rhs=act_in[:, c, :], start=(c == 0), stop=(c == cin - 1))
                    evict(m, ps[:, :])

            # ---- activation-stationary GEMM: act_feat [P, cin, T] @ W[cin*P, dout]
            #      -> token-major out, evict(tt, nb, psum[P, 512]) ----
            def gemm_as(w_dram, act_feat, cin, dout, evict):
                slabs = []
                for c in range(cin):
                    wsl = wslp.tile([P, dout], bf16, tag="wslab", name="wsl")
                    nc.sync.dma_start(out=wsl, in_=w_dram[c * P:(c + 1) * P, :])
                    slabs.append(wsl)
                for tt in range(TQ):
                    for nb in range(dout // 512):
                        ps = psA.tile([P, 512], f32, tag="gps", name="gps")
                        for c in range(cin):
                            nc.tensor.matmul(ps[:, :], lhsT=act_feat[:, c, tt * P:(tt + 1) * P],
                                             rhs=slabs[c][:, nb * 512:(nb + 1) * 512],
                                             start=(c == 0), stop=(c == cin - 1))
                        evict(tt, nb, ps[:, :])

            # ================= embedding =================
            seq_sb = cst.tile([V, T], bf16, tag="seq")
            nc.sync.dma_start(out=seq_sb, in_=seq_oh[:, :])
            te_sb = cst.tile([V, NE], bf16, tag="te")
            nc.sync.dma_start(out=te_sb, in_=te_w[:, :])
            pos_sb = cst.tile([P, TQ, NE], bf16, tag="pos")
            nc.sync.dma_start(out=pos_sb, in_=pos_e[:, :].rearrange("(tt p) n -> p tt n", p=P))

            emb = acts.tile([P, TQ, NE], bf16, tag="x", bufs=2)
            for tt in range(TQ):
                for nb in range(2):
                    ps = psA.tile([P, 512], f32, tag="gps", name="gps")
                    nc.tensor.matmul(ps[:, :], lhsT=seq_sb[:, tt * P:(tt + 1) * P],
                                     rhs=te_sb[:, nb * 512:(nb + 1) * 512], start=True, stop=True)
                    nc.vector.tensor_tensor(out=emb[:, tt, nb * 512:(nb + 1) * 512], in0=ps[:, :],
                                            in1=pos_sb[:, tt, nb * 512:(nb + 1) * 512], op=OP.add)

            e_n = acts.tile([P, TQ, NE], bf16, tag="h_tok")
            ln_tok(emb, None, None, e_n)
            e_nf = acts.tile([P, CD, T], bf16, tag="h_feat")
            t2f(e_n, e_nf)

            fbr = ldrep(fc_b[:, :])
            x = acts.tile([P, TQ, DM], bf16, tag="x", bufs=2)

            def fc_evict(tt, nb, ps_ap):
                nc.vector.tensor_tensor(out=x[:, tt, nb * 512:(nb + 1) * 512], in0=ps_ap,
                                        in1=fbr[:, nb * 512:(nb + 1) * 512], op=OP.add)

            gemm_as(fc_w[:, :], e_nf, CD, DM, fc_evict)

            # ================= transformer layers =================
            for l in range(L):
                l1g = ldrep(ln1_g[l]); l1b = ldrep(ln1_b[l])
                h_tok = acts.tile([P, TQ, DM], bf16, tag="h_tok")
                ln_tok(x, l1g, l1b, h_tok)
                h = acts.tile([P, CD, T], bf16, tag="h_feat")
                t2f(h_tok, h)

                qb = ldvec(wq_b[l], CD); kb = ldvec(wk_b[l], CD)
                vb = ldvec(wv_b[l], CD)
                q = acts.tile([P, CD, T], bf16, tag="q")
                k = acts.tile([P, CD, T], bf16, tag="k")
                v_tok = acts.tile([P, TQ, H * DVh], bf16, tag="vtok")
                ctx = acts.tile([P, CD, T], bf16, tag="ctx")

                qsl, ksl, vsl = [], [], []
                for c in range(CD):
                    for wdram, lst in ((wq_w, qsl), (wk_w, ksl), (wv_w, vsl)):
                        wsl = wslp.tile([P, DM], bf16, tag="wslab", name="wsl")
                        nc.sync.dma_start(out=wsl, in_=wdram[l, c * P:(c + 1) * P, :])
                        lst.append(wsl)

                def qk_chunk(m):
                    for slabs, dst, bias in ((qsl, q, qb), (ksl, k, kb)):
                        ps = psA.tile([P, T], f32, tag="gps", name="gps")
                        for c in range(CD):
                            nc.tensor.matmul(ps[:, :], lhsT=slabs[c][:, m * P:(m + 1) * P],
                                             rhs=h[:, c, :], start=(c == 0), stop=(c == CD - 1))
                        nc.vector.tensor_scalar_add(dst[:, m, :], ps[:, :], bias[:, m:m + 1])

                def v_block(i):
                    tt, nb = i % 2, i // 2
                    ps = psA.tile([P, 512], f32, tag="gps", name="gps")
                    for c in range(CD):
                        nc.tensor.matmul(ps[:, :], lhsT=h[:, c, tt * P:(tt + 1) * P],
                                         rhs=vsl[c][:, nb * 512:(nb + 1) * 512],
                                         start=(c == 0), stop=(c == CD - 1))
                    nc.scalar.activation(out=v_tok[:, tt, nb * 512:(nb + 1) * 512], in_=ps[:, :],
                                         func=AF.Copy, bias=0.0, scale=1.0)

                def attn_pair(hp):
                    atTs = []
                    for hl in range(2):
                        o = hl * 64
                        scT = psA.tile([P, TQ, T], f32, tag="gps", name="scT")
                        for kt in range(TQ):
                            nc.tensor.matmul(scT[:, kt, :], lhsT=k[o:o + 64, hp, kt * P:(kt + 1) * P],
                                             rhs=q[o:o + 64, hp, :], start=True, stop=True)
                        eT = sm.tile([P, TQ, T], bf16, tag="eT", name="eT", bufs=5)
                        nc.scalar.activation(out=eT[:, :, :], in_=scT[:, :, :], func=AF.Exp,
                                             bias=0.0, scale=SCALE)
                        for kt in range(TQ):
                            nc.vector.tensor_scalar_mul(eT[:, kt, :], eT[:, kt, :], mask_sb[:, kt:kt + 1])
                        ssum = psB.tile([1, T], f32, tag="ssum", name="ssum")
                        for kt in range(TQ):
                            nc.tensor.matmul(ssum[:, :], lhsT=onesk[:, :], rhs=eT[:, kt, :],
                                             start=(kt == 0), stop=(kt == TQ - 1))
                        rsf = sm.tile([1, T], f32, tag="rsf", name="rsf", bufs=4)
                        nc.vector.reciprocal_approx_fast(out=rsf[:, :], in_=ssum[:, :])
                        rs = sm.tile([1, T], bf16, tag="rs", name="rs", bufs=4)
                        nc.vector.tensor_copy(rs[:, :], rsf[:, :])
                        rb = psB.tile([P, T], f32, tag="rb", name="rb")
                        nc.tensor.matmul(rb[:, :], lhsT=ones1[:, :], rhs=rs[:, :], start=True, stop=True)
                        atT = sm.tile([P, TQ, T], bf16, tag="atT", name="atT", bufs=5)
                        for kt in range(TQ):
                            nc.vector.tensor_tensor(out=atT[:, kt, :], in0=eT[:, kt, :], in1=rb[:, :], op=OP.mult)
                        atTs.append(atT)
                    cp = psB.tile([P, T], f32, tag="cps", name="cps")
                    for hl in range(2):
                        dv0 = (2 * hp + hl) * DVh
                        for kt in range(TQ):
                            nc.tensor.matmul(cp[hl * 64:hl * 64 + 64, :],
                                             lhsT=v_tok[:, kt, dv0:dv0 + DVh],
                                             rhs=atTs[hl][:, kt, :], start=(kt == 0), stop=(kt == TQ - 1),
                                             tile_position=(0, hl * 64))
                    nc.vector.tensor_scalar_add(ctx[:, hp, :], cp[:, :], vb[:, hp:hp + 1])

                # interleave: dense qk/v chunks between attention pair chains
                for hp in range(CD):
                    qk_chunk(hp)
                    if hp < 4:
                        v_block(hp)
                    if hp >= 2:
                        attn_pair(hp - 2)
                attn_pair(CD - 2)
                attn_pair(CD - 1)

                # ---- attention out (act-stationary) + residual h_tok ----
                abr = ldrep(ao_b[l])
                x2 = acts.tile([P, TQ, DM], bf16, tag="x2")

                def ao_evict(tt, nb, ps_ap):
                    t_ = sm.tile([P, 512], bf16, tag="aot", name="aot")
                    nc.vector.tensor_tensor(out=t_[:, :], in0=ps_ap,
                                            in1=h_tok[:, tt, nb * 512:(nb + 1) * 512], op=OP.add)
                    nc.vector.tensor_tensor(out=x2[:, tt, nb * 512:(nb + 1) * 512], in0=t_[:, :],
                                            in1=abr[:, nb * 512:(nb + 1) * 512], op=OP.add)

                gemm_as(ao_w[l], ctx, CD, DM, ao_evict)

                h2_tok = acts.tile([P, TQ, DM], bf16, tag="h2_tok")
                ln_tok(x2, None, None, h2_tok)
                h2 = acts.tile([P, CD, T], bf16, tag="h2_feat")
                t2f(h2_tok, h2)

                # ---- FFN ----
                f1b = ldvec(f1_b[l], CF)
                h1 = acts.tile([P, CF, T], bf16, tag="h1")
                for blk in range(4):
                    def f1_evict(m, ps_ap, blk=blk):
                        mg = blk * 8 + m
                        nc.scalar.activation(out=h1[:, mg, :], in_=ps_ap, func=AF.Gelu,
                                             bias=f1b[:, mg:mg + 1], scale=1.0)
                    gemm_ws(f1_w[l][:, blk * 1024:(blk + 1) * 1024], h2, CD, CD, f1_evict)

                fbr2 = ldrep(f2_b[l])
                xn = acts.tile([P, TQ, DM], bf16, tag="x", bufs=2)

                def f2_evict(tt, nb, ps_ap):
                    nc.vector.tensor_tensor(out=xn[:, tt, nb * 512:(nb + 1) * 512], in0=ps_ap,
                                            in1=fbr2[:, nb * 512:(nb + 1) * 512], op=OP.add)

                gemm_as(f2_w[l], h1, CF, DM, f2_evict)
                x = xn

            # ================= final LN, gather, head =================
            xf = acts.tile([P, TQ, DM], bf16, tag="h_tok")
            ln_tok(x, None, None, xf)

            poh = cst.tile([P, TQ, NM], bf16, tag="poh")
            nc.sync.dma_start(out=poh, in_=pos_oh[:, :].rearrange("(tt p) m -> p tt m", p=P))
            sel_ps = psB.tile([P, CD, NM], f32, tag="sc", name="selps")
            for c in range(CD):
                for tt in range(TQ):
                    nc.tensor.matmul(sel_ps[:, c, :], lhsT=xf[:, tt, c * P:(c + 1) * P],
                                     rhs=poh[:, tt, :], start=(tt == 0), stop=(tt == TQ - 1))
            sel = sm.tile([P, CD, NM], bf16, tag="sel")
            nc.scalar.activation(out=sel[:, :, :], in_=sel_ps[:, :, :], func=AF.Copy, bias=0.0, scale=1.0)

            linb = ldvec(lin_b[:, :], CD)
            lslabs = []
            for c in range(CD):
                wsl = wslp.tile([P, DM], bf16, tag="wslab", name="wsl")
                nc.sync.dma_start(out=wsl, in_=lin_w[c * P:(c + 1) * P, :])
                lslabs.append(wsl)
            hg = sm.tile([P, CD, NM], bf16, tag="hg")
            for m in range(CD):
                lin_ps = psA.tile([P, NM], f32, tag="gps", name="lps")
                for c in range(CD):
                    nc.tensor.matmul(lin_ps[:, :], lhsT=lslabs[c][:, m * P:(m + 1) * P],
                                     rhs=sel[:, c, :], start=(c == 0), stop=(c == CD - 1))
                nc.scalar.activation(out=hg[:, m, :], in_=lin_ps[:, :], func=AF.Gelu,
                                     bias=linb[:, m:m + 1], scale=1.0)

            hw_sb = cst.tile([P, CD, V], bf16, tag="hw")
            nc.sync.dma_start(out=hw_sb, in_=head_w[:, :].rearrange("(c p) v -> p c v", p=P))
            lg_ps = psB.tile([V, NM], f32, tag="ssum", name="lgps")
            for c in range(CD):
                nc.tensor.matmul(lg_ps[:, :], lhsT=hw_sb[:, c, :], rhs=hg[:, c, :],
                                 start=(c == 0), stop=(c == CD - 1))
            lgT = sm.tile([V, NM], f32, tag="lgT")
            nc.vector.tensor_copy(lgT[:, :], lg_ps[:, :])
            nc.sync.dma_start(out=logits_out[:, :], in_=lgT[:, :])

    nc.finalize()
    return nc


def _col_layout(vec, nch):
    return np.ascontiguousarray(np.asarray(vec, np.float32).reshape(nch, P).T)


def _rep(vec):
    v = np.asarray(vec).astype(BF)
    return np.ascontiguousarray(np.broadcast_to(v[None, :], (P, v.shape[0])))


def prepare_inputs(inputs):
    f = {k: np.asarray(v) for k, v in inputs.items()}
    # fold LN gains into the following linear layers (all fp32 on host):
    # fc' = diag(norm_g) @ fc_w, fc_b' = fc_b + norm_b @ fc_w
    fc_w_f = np.asarray(f["norm_g"], np.float32)[:, None] * np.asarray(f["fc_w"], np.float32)
    fc_b_f = np.asarray(f["fc_b"], np.float32) + np.asarray(f["norm_b"], np.float32) @ np.asarray(f["fc_w"], np.float32)
    f1_w_f = np.asarray(f["ln2_g"], np.float32)[:, :, None] * np.asarray(f["f1_w"], np.float32)
    f1_b_f = np.asarray(f["f1_b"], np.float32) + np.einsum(
        "ld,ldo->lo", np.asarray(f["ln2_b"], np.float32), np.asarray(f["f1_w"], np.float32))
    lin_w_f = np.asarray(f["lnf_g"], np.float32)[:, None] * np.asarray(f["lin_w"], np.float32)
    lin_b_f = np.asarray(f["lin_b"], np.float32) + np.asarray(f["lnf_b"], np.float32) @ np.asarray(f["lin_w"], np.float32)
    wmap = {
        "te_w": f["tok_embed"].astype(BF),
        "pos_e": f["pos_embed"][0].astype(BF),
        "fc_w": fc_w_f.astype(BF),
        "wq_w": f["wq_w"].astype(BF),
        "wk_w": f["wk_w"].astype(BF),
        "wv_w": f["wv_w"].astype(BF),
        "ao_w": f["ao_w"].astype(BF),
        "f1_w": f1_w_f.astype(BF),
        "f2_w": f["f2_w"].astype(BF),
        "lin_w": lin_w_f.astype(BF),
        "head_w": f["head_w"].astype(BF),
                "fc_b": _rep(fc_b_f),
        "lin_b": _col_layout(lin_b_f, CD),
        "ln1_g": np.stack([_rep(f["ln1_g"][l]) for l in range(L)]),
        "ln1_b": np.stack([_rep(f["ln1_b"][l]) for l in range(L)]),
        "ao_b": np.stack([_rep(f["ao_b"][l]) for l in range(L)]),
        "f2_b": np.stack([_rep(f["f2_b"][l]) for l in range(L)]),
        "wq_b": np.stack([_col_layout(f["wq_b"][l], CD) for l in range(L)]),
        "wk_b": np.stack([_col_layout(f["wk_b"][l], CD) for l in range(L)]),
        "wv_b": np.stack([_col_layout(f["wv_b"][l], CD) for l in range(L)]),
        "f1_b": np.stack([_col_layout(f1_b_f[l], CF) for l in range(L)]),
    }
    seqs = np.asarray(f["masked_seqs"]).astype(np.int64)
    poss = np.asarray(f["masked_pos"]).astype(np.int64)
    in_maps = []
    for b in range(B):
        seq_oh = (seqs[b][None, :] == np.arange(V)[:, None]).astype(BF)
        pos_oh = (poss[b][None, :] == np.arange(T)[:, None]).astype(BF)
        maskrow = np.where(seqs[b] == 0, 0.0, 1.0).astype(np.float32)
        maskc = np.ascontiguousarray(maskrow.reshape(TQ, P).T)  # [k%128, k//128]
        in_maps.append({"seq_oh": seq_oh, "pos_oh": np.ascontiguousarray(pos_oh),
                        "maskc": maskc, **wmap})
    return in_maps


_GRAPH_CACHE = {}


def run(inputs, trace=False):
    in_maps = prepare_inputs(inputs)
    if "nc" not in _GRAPH_CACHE:
        _GRAPH_CACHE["nc"] = build_graph()
    nc = _GRAPH_CACHE["nc"]
    res = run_bass_kernel_spmd(nc, in_maps, core_ids=list(range(B)), trace=trace)
    logits = np.stack([np.ascontiguousarray(np.asarray(r["logitsT"]).T) for r in res.results])
    lg = logits.astype(np.float32)
    mx = lg.max(-1, keepdims=True)
    logp = lg - mx - np.log(np.exp(lg - mx).sum(-1, keepdims=True))
    tok = np.asarray(inputs["masked_tokens"]).astype(np.int64)
    picked = np.take_along_axis(logp, tok[:, :, None], axis=-1)
    loss = np.float32(-picked.mean())
    return (logits, loss), res


def kernel(**inputs):
    (logits, loss), _ = run(inputs, trace=False)
    return logits, loss


# revision 23
# speedup vs baseline: 1.3308x; 1.3308x over previous
"""BERT-style MLM forward on 8 TRN2 NeuronCores.

Data-parallel over batch (B=8 -> 1 sequence per core, no collectives).

v2 design:
- Residual stream is TOKEN-major [t%128, t//128, dm]: LayerNorm runs entirely
  on the Vector engine (bn_stats/bn_aggr + per-partition tensor_scalar), no
  TensorE stats matmuls and no cross-partition broadcasts.
- Q/K/fc1 GEMMs are weights-stationary feature-major (outputs land feature-
  major for attention / fc2). V/AO/fc2/fc GEMMs are activation-stationary
  (lhsT = activation chunk, rhs = weight slab streamed at N=512), producing
  token-major outputs directly.
- Attention computes scores TRANSPOSED ([k, q] = K^T-stationary), so softmax
  normalization is a ones-matmul partition reduction and attn^T feeds the ctx
  matmul directly -- no per-head PE transposes.
- bf16 matmul operands, fp32 PSUM/statistics. One-hot matmuls for embedding
  and masked-position gather (one-hots built on host). Loss replicated on
  host from device logits.
"""
import sys

try:
    import concourse.bass as bass  # noqa: F401
except ImportError:
    sys.path.insert(0, "/opt/trn_rl_repo")

import ml_dtypes
import numpy as np

import concourse.bass as bass
import concourse.bass_utils as _bu
import concourse.mybir as mybir
from concourse import bacc, tile
from concourse.bass_utils import run_bass_kernel_spmd
from concourse.masks import make_identity



B, T, NM = 8, 256, 40
V, NE, DM, DK, DVh, H, DFF, L = 30, 1024, 1024, 64, 64, 16, 4096, 8
EPS = 1e-5
P = 128
CD = DM // P   # 8
CF = DFF // P  # 32
TQ = T // P    # 2
SCALE = float(1.0 / np.sqrt(DK).astype(np.float32))

bf16 = mybir.dt.bfloat16
f32 = mybir.dt.float32
BF = ml_dtypes.bfloat16
AF = mybir.ActivationFunctionType
OP = mybir.AluOpType
AX = mybir.AxisListType


def build_graph():
    nc = bacc.Bacc()

    def param(name, shape, dt=bf16):
        return nc.declare_dram_parameter(name, list(shape), dt, isOutput=False)

    # per-core tensors
    seq_oh = param("seq_oh", (V, T))            # one-hot tokens [v, t]
    pos_oh = param("pos_oh", (T, NM))           # one-hot masked positions
    maskc = param("maskc", (P, TQ), f32)        # 0/1 key mask, [k%128, k//128]
    maskcb = param("maskcb", (P, TQ), bf16)     # same mask, bf16 (matmul lhsT)
    rsel_p = param("rsel", (1, 2, P), bf16)     # half-partition selector rows
    # weights
    te_w = param("te_w", (V, NE))
    pos_e = param("pos_e", (T, NE))             # pos_embed[0], token-major
    fc_w = param("fc_w", (NE, DM))
    wq_w = param("wq_w", (L, DM, H * DK))
    wk_w = param("wk_w", (L, DM, H * DK))
    wv_w = param("wv_w", (L, DM, H * DVh))
    ao_w = param("ao_w", (L, H * DVh, DM))
    f1_w = param("f1_w", (L, DM, DFF))
    f2_w = param("f2_w", (L, DFF, DM))
    lin_w = param("lin_w", (DM, DM))
    head_w = param("head_w", (DM, V))
    # per-partition column layouts [P, nchunks] (feature-major eviction biases)
    wq_b = param("wq_b", (L, P, CD), f32)
    wk_b = param("wk_b", (L, P, CD), f32)
    wv_b = param("wv_b", (L, P, CD), f32)
    f1_b = param("f1_b", (L, P, CF), f32)
    lin_b = param("lin_b", (P, CD), f32)
    # replicated row layouts [P, DM] (token-major: per-feature on free axis)
    fc_b = param("fc_b", (P, DM))
    ln1_g = param("ln1_g", (L, P, DM))
    ln1_b = param("ln1_b", (L, P, DM))
    ao_b = param("ao_b", (L, P, DM))
    f2_b = param("f2_b", (L, P, DM))
    logits_out = nc.declare_dram_parameter("logitsT", [V, NM], f32, isOutput=True)

    with tile.TileContext(nc) as tc:
        with tc.tile_pool(name="cst", bufs=1) as cst, \
             tc.tile_pool(name="acts", bufs=1) as acts, \
             tc.tile_pool(name="wsl", bufs=36) as wslp, \
             tc.tile_pool(name="reps", bufs=6) as reps, \
             tc.tile_pool(name="sm", bufs=2) as sm, \
             tc.tile_pool(name="psA", bufs=4, space="PSUM") as psA, \
             tc.tile_pool(name="psB", bufs=1, space="PSUM") as psB:

            ident = cst.tile([P, P], bf16, tag="ident")
            make_identity(nc, ident)
            onesk = cst.tile([P, 1], bf16, tag="onesk")
            nc.vector.memset(onesk, 1.0)
            ones1 = cst.tile([1, P], bf16, tag="ones1")
            nc.vector.memset(ones1, 1.0)
            eps_t = cst.tile([P, 1], f32, tag="eps")
            nc.vector.memset(eps_t, EPS)
            mask_sb = cst.tile([P, TQ], f32, tag="mask")
            nc.sync.dma_start(out=mask_sb, in_=maskc[:, :])
            mask_bf = cst.tile([P, TQ], bf16, tag="maskb")
            nc.sync.dma_start(out=mask_bf, in_=maskcb[:, :])
            rsel = cst.tile([1, 2, P], bf16, tag="rsel")
            nc.sync.dma_start(out=rsel, in_=rsel_p[:, :, :])

            def ldvec(ap2d, nchunk):
                t = sm.tile([P, nchunk], f32, tag="vecs", name="vec")
                nc.sync.dma_start(out=t, in_=ap2d)
                return t

            def ldrep(ap2d):
                t = reps.tile([P, DM], bf16, tag="rep", name="rep")
                nc.sync.dma_start(out=t, in_=ap2d)
                return t

            # ---- token-major layernorm: all-DVE + tiny ACT sqrt ----
            def ln_tok(x, g_rep, b_rep, out):
                for tt in range(TQ):
                    stats = sm.tile([P, 2, 6], f32, tag="ln_st", name="ln_st")
                    for sg in range(2):
                        nc.vector.bn_stats(out=stats[:, sg, :], in_=x[:, tt, sg * 512:(sg + 1) * 512])
                    mv = sm.tile([P, 2], f32, tag="ln_mv", name="ln_mv")
                    nc.vector.bn_aggr(out=mv[:, :], in_=stats[:, :, :])
                    std = sm.tile([P, 1], f32, tag="ln_sd", name="ln_sd")
                    nc.scalar.activation(out=std[:, :], in_=mv[:, 1:2], func=AF.Sqrt,
                                         bias=eps_t[:, :], scale=1.0)
                    nc.vector.reciprocal(out=std[:, :], in_=std[:, :])
                    if g_rep is None:
                        nc.vector.tensor_scalar(out=out[:, tt, :], in0=x[:, tt, :],
                                                scalar1=mv[:, 0:1], scalar2=std[:, :],
                                                op0=OP.subtract, op1=OP.mult)
                    else:
                        tmp = sm.tile([P, DM], bf16, tag="ln_t", name="ln_t")
                        nc.vector.tensor_scalar(out=tmp[:, :], in0=x[:, tt, :],
                                                scalar1=mv[:, 0:1], scalar2=std[:, :],
                                                op0=OP.subtract, op1=OP.mult)
                        nc.vector.tensor_tensor(out=tmp[:, :], in0=tmp[:, :], in1=g_rep[:, :], op=OP.mult)
                        nc.vector.tensor_tensor(out=out[:, tt, :], in0=tmp[:, :], in1=b_rep[:, :], op=OP.add)

            # ---- token-major -> feature-major transpose ----
            def t2f(x_tok, out_feat):
                for cg in range(2):
                    tp = psB.tile([P, 8, P], bf16, tag="sc", name="tp")
                    for ci in range(4):
                        c = cg * 4 + ci
                        for tt in range(TQ):
                            nc.tensor.transpose(tp[:, ci * 2 + tt, :], x_tok[:, tt, c * P:(c + 1) * P],
                                                ident[:, :])
                    for ci in range(4):
                        c = cg * 4 + ci
                        nc.vector.tensor_copy(out_feat[:, c, :], tp[:, ci * 2:ci * 2 + 2, :])

            # ---- weights-stationary feature-major GEMM (m-outer, slabs resident) ----
            def gemm_ws(w_dram, act_in, cin, mout, evict):
                slabs = []
                for c in range(cin):
                    wsl = wslp.tile([P, mout * P], bf16, tag="wslab", name="wsl")
                    nc.sync.dma_start(out=wsl, in_=w_dram[c * P:(c + 1) * P, :])
                    slabs.append(wsl)
                for m in range(mout):
                    ps = psA.tile([P, T], f32, tag="gps", name="gps")
                    for c in range(cin):
                        nc.tensor.matmul(ps[:, :], lhsT=slabs[c][:, m * P:(m + 1) * P],
                                         rhs=act_in[:, c, :], start=(c == 0), stop=(c == cin - 1))
                    evict(m, ps[:, :])

            # ---- activation-stationary GEMM: act_feat [P, cin, T] @ W[cin*P, dout]
            #      -> token-major out, evict(tt, nb, psum[P, 512]) ----
            def gemm_as(w_dram, act_feat, cin, dout, evict):
                slabs = []
                for c in range(cin):
                    wsl = wslp.tile([P, dout], bf16, tag="wslab", name="wsl")
                    nc.sync.dma_start(out=wsl, in_=w_dram[c * P:(c + 1) * P, :])
                    slabs.append(wsl)
                for tt in range(TQ):
                    for nb in range(dout // 512):
                        ps = psA.tile([P, 512], f32, tag="gps", name="gps")
                        for c in range(cin):
                            nc.tensor.matmul(ps[:, :], lhsT=act_feat[:, c, tt * P:(tt + 1) * P],
                                             rhs=slabs[c][:, nb * 512:(nb + 1) * 512],
                                             start=(c == 0), stop=(c == cin - 1))
                        evict(tt, nb, ps[:, :])

            # ================= embedding =================
            seq_sb = cst.tile([V, T], bf16, tag="seq")
            nc.sync.dma_start(out=seq_sb, in_=seq_oh[:, :])
            te_sb = cst.tile([V, NE], bf16, tag="te")
            nc.sync.dma_start(out=te_sb, in_=te_w[:, :])
            pos_sb = cst.tile([P, TQ, NE], bf16, tag="pos")
            nc.sync.dma_start(out=pos_sb, in_=pos_e[:, :].rearrange("(tt p) n -> p tt n", p=P))

            emb = acts.tile([P, TQ, NE], bf16, tag="x", bufs=2)
            for tt in range(TQ):
                for nb in range(2):
                    ps = psA.tile([P, 512], f32, tag="gps", name="gps")
                    nc.tensor.matmul(ps[:, :], lhsT=seq_sb[:, tt * P:(tt + 1) * P],
                                     rhs=te_sb[:, nb * 512:(nb + 1) * 512], start=True, stop=True)
                    nc.vector.tensor_tensor(out=emb[:, tt, nb * 512:(nb + 1) * 512], in0=ps[:, :],
                                            in1=pos_sb[:, tt, nb * 512:(nb + 1) * 512], op=OP.add)

            e_n = acts.tile([P, TQ, NE], bf16, tag="h_tok")
            ln_tok(emb, None, None, e_n)
            e_nf = acts.tile([P, CD, T], bf16, tag="h_feat")
            t2f(e_n, e_nf)

            fbr = ldrep(fc_b[:, :])
            x = acts.tile([P, TQ, DM], bf16, tag="x", bufs=2)

            def fc_evict(tt, nb, ps_ap):
                nc.vector.tensor_tensor(out=x[:, tt, nb * 512:(nb + 1) * 512], in0=ps_ap,
                                        in1=fbr[:, nb * 512:(nb + 1) * 512], op=OP.add)

            gemm_as(fc_w[:, :], e_nf, CD, DM, fc_evict)

            # ================= transformer layers =================
            for l in range(L):
                l1g = ldrep(ln1_g[l]); l1b = ldrep(ln1_b[l])
                h_tok = acts.tile([P, TQ, DM], bf16, tag="h_tok")
                ln_tok(x, l1g, l1b, h_tok)
                h = acts.tile([P, CD, T], bf16, tag="h_feat")
                t2f(h_tok, h)

                qb = ldvec(wq_b[l], CD); kb = ldvec(wk_b[l], CD)
                vb = ldvec(wv_b[l], CD)
                q = acts.tile([P, CD, T], bf16, tag="q")
                k = acts.tile([P, CD, T], bf16, tag="k")
                v_tok = acts.tile([P, TQ, H * DVh], bf16, tag="vtok")
                ctx = acts.tile([P, CD, T], bf16, tag="ctx")

                qsl, ksl, vsl = [], [], []
                for c in range(CD):
                    for wdram, lst in ((wq_w, qsl), (wk_w, ksl), (wv_w, vsl)):
                        wsl = wslp.tile([P, DM], bf16, tag="wslab", name="wsl")
                        nc.sync.dma_start(out=wsl, in_=wdram[l, c * P:(c + 1) * P, :])
                        lst.append(wsl)

                def qk_chunk(m):
                    for slabs, dst, bias in ((qsl, q, qb), (ksl, k, kb)):
                        ps = psA.tile([P, T], f32, tag="gps", name="gps")
                        for c in range(CD):
                            nc.tensor.matmul(ps[:, :], lhsT=slabs[c][:, m * P:(m + 1) * P],
                                             rhs=h[:, c, :], start=(c == 0), stop=(c == CD - 1))
                        nc.vector.tensor_scalar_add(dst[:, m, :], ps[:, :], bias[:, m:m + 1])

                def v_block(i):
                    tt, nb = i % 2, i // 2
                    ps = psA.tile([P, 512], f32, tag="gps", name="gps")
                    for c in range(CD):
                        nc.tensor.matmul(ps[:, :], lhsT=h[:, c, tt * P:(tt + 1) * P],
                                         rhs=vsl[c][:, nb * 512:(nb + 1) * 512],
                                         start=(c == 0), stop=(c == CD - 1))
                    nc.scalar.activation(out=v_tok[:, tt, nb * 512:(nb + 1) * 512], in_=ps[:, :],
                                         func=AF.Copy, bias=0.0, scale=mask_sb[:, tt:tt + 1])

                def attn_pair(hp):
                    cp = psB.tile([P, T], f32, tag="cps", name="cps")
                    rbp = psB.tile([P, T], f32, tag="rb", name="rbp")
                    for hl in range(2):
                        o = hl * 64
                        scT = psA.tile([P, TQ, T], f32, tag="gps", name="scT")
                        for kt in range(TQ):
                            nc.tensor.matmul(scT[:, kt, :], lhsT=k[o:o + 64, hp, kt * P:(kt + 1) * P],
                                             rhs=q[o:o + 64, hp, :], start=True, stop=True)
                        eT = sm.tile([P, TQ, T], bf16, tag="eT", name="eT", bufs=5)
                        nc.scalar.activation(out=eT[:, :, :], in_=scT[:, :, :], func=AF.Exp,
                                             bias=0.0, scale=SCALE)
                        dv0 = (2 * hp + hl) * DVh
                        for kt in range(TQ):
                            nc.tensor.matmul(cp[hl * 64:hl * 64 + 64, :],
                                             lhsT=v_tok[:, kt, dv0:dv0 + DVh],
                                             rhs=eT[:, kt, :], start=(kt == 0), stop=(kt == TQ - 1),
                                             tile_position=(0, hl * 64))
                        ssum = psB.tile([1, T], f32, tag="ssum", name="ssum")
                        for kt in range(TQ):
                            nc.tensor.matmul(ssum[:, :], lhsT=mask_bf[:, kt:kt + 1], rhs=eT[:, kt, :],
                                             start=(kt == 0), stop=(kt == TQ - 1))
                        rsf = sm.tile([1, T], f32, tag="rsf", name="rsf", bufs=4)
                        nc.vector.reciprocal_approx_fast(out=rsf[:, :], in_=ssum[:, :])
                        rsb = sm.tile([1, T], bf16, tag="rsb", name="rsb", bufs=4)
                        nc.vector.tensor_copy(rsb[:, :], rsf[:, :])
                        nc.tensor.matmul(rbp[:, :], lhsT=rsel[:, hl, :], rhs=rsb[:, :],
                                         start=(hl == 0), stop=(hl == 1))
                    rbs = sm.tile([P, T], bf16, tag="rbs", name="rbs", bufs=3)
                    nc.vector.tensor_copy(rbs[:, :], rbp[:, :])
                    cn = sm.tile([P, T], bf16, tag="cn", name="cn", bufs=3)
                    nc.vector.tensor_tensor(out=cn[:, :], in0=cp[:, :], in1=rbs[:, :], op=OP.mult)
                    nc.vector.tensor_scalar_add(ctx[:, hp, :], cn[:, :], vb[:, hp:hp + 1])

                # interleave: dense qk/v chunks between attention pair chains
                for hp in range(CD):
                    qk_chunk(hp)
                    if hp < 4:
                        v_block(hp)
                    if hp >= 2:
                        attn_pair(hp - 2)
                attn_pair(CD - 2)
                attn_pair(CD - 1)

                # ---- attention out (act-stationary) + residual h_tok ----
                abr = ldrep(ao_b[l])
                x2 = acts.tile([P, TQ, DM], bf16, tag="x2")

                def ao_evict(tt, nb, ps_ap):
                    t_ = sm.tile([P, 512], bf16, tag="aot", name="aot")
                    nc.vector.tensor_tensor(out=t_[:, :], in0=ps_ap,
                                            in1=h_tok[:, tt, nb * 512:(nb + 1) * 512], op=OP.add)
                    nc.vector.tensor_tensor(out=x2[:, tt, nb * 512:(nb + 1) * 512], in0=t_[:, :],
                                            in1=abr[:, nb * 512:(nb + 1) * 512], op=OP.add)

                gemm_as(ao_w[l], ctx, CD, DM, ao_evict)

                h2_tok = acts.tile([P, TQ, DM], bf16, tag="h2_tok")
                ln_tok(x2, None, None, h2_tok)
                h2 = acts.tile([P, CD, T], bf16, tag="h2_feat")
                t2f(h2_tok, h2)

                # ---- FFN ----
                f1b = ldvec(f1_b[l], CF)
                h1 = acts.tile([P, CF, T], bf16, tag="h1")
                for blk in range(4):
                    def f1_evict(m, ps_ap, blk=blk):
                        mg = blk * 8 + m
                        nc.scalar.activation(out=h1[:, mg, :], in_=ps_ap, func=AF.Gelu,
                                             bias=f1b[:, mg:mg + 1], scale=1.0)
                    gemm_ws(f1_w[l][:, blk * 1024:(blk + 1) * 1024], h2, CD, CD, f1_evict)

                fbr2 = ldrep(f2_b[l])
                xn = acts.tile([P, TQ, DM], bf16, tag="x", bufs=2)

                def f2_evict(tt, nb, ps_ap):
                    nc.vector.tensor_tensor(out=xn[:, tt, nb * 512:(nb + 1) * 512], in0=ps_ap,
                                            in1=fbr2[:, nb * 512:(nb + 1) * 512], op=OP.add)

                gemm_as(f2_w[l], h1, CF, DM, f2_evict)
                x = xn

            # ================= final LN, gather, head =================
            xf = acts.tile([P, TQ, DM], bf16, tag="h_tok")
            ln_tok(x, None, None, xf)

            poh = cst.tile([P, TQ, NM], bf16, tag="poh")
            nc.sync.dma_start(out=poh, in_=pos_oh[:, :].rearrange("(tt p) m -> p tt m", p=P))
            sel_ps = psB.tile([P, CD, NM], f32, tag="sc", name="selps")
            for c in range(CD):
                for tt in range(TQ):
                    nc.tensor.matmul(sel_ps[:, c, :], lhsT=xf[:, tt, c * P:(c + 1) * P],
                                     rhs=poh[:, tt, :], start=(tt == 0), stop=(tt == TQ - 1))
            sel = sm.tile([P, CD, NM], bf16, tag="sel")
            nc.scalar.activation(out=sel[:, :, :], in_=sel_ps[:, :, :], func=AF.Copy, bias=0.0, scale=1.0)

            linb = ldvec(lin_b[:, :], CD)
            lslabs = []
            for c in range(CD):
                wsl = wslp.tile([P, DM], bf16, tag="wslab", name="wsl")
                nc.sync.dma_start(out=wsl, in_=lin_w[c * P:(c + 1) * P, :])
                lslabs.append(wsl)
            hg = sm.tile([P, CD, NM], bf16, tag="hg")
            for m in range(CD):
                lin_ps = psA.tile([P, NM], f32, tag="gps", name="lps")
                for c in range(CD):
                    nc.tensor.matmul(lin_ps[:, :], lhsT=lslabs[c][:, m * P:(m + 1) * P],
                                     rhs=sel[:, c, :], start=(c == 0), stop=(c == CD - 1))
                nc.scalar.activation(out=hg[:, m, :], in_=lin_ps[:, :], func=AF.Gelu,
                                     bias=linb[:, m:m + 1], scale=1.0)

            hw_sb = cst.tile([P, CD, V], bf16, tag="hw")
            nc.sync.dma_start(out=hw_sb, in_=head_w[:, :].rearrange("(c p) v -> p c v", p=P))
            lg_ps = psB.tile([V, NM], f32, tag="ssum", name="lgps")
            for c in range(CD):
                nc.tensor.matmul(lg_ps[:, :], lhsT=hw_sb[:, c, :], rhs=hg[:, c, :],
                                 start=(c == 0), stop=(c == CD - 1))
            lgT = sm.tile([V, NM], f32, tag="lgT")
            nc.vector.tensor_copy(lgT[:, :], lg_ps[:, :])
            nc.sync.dma_start(out=logits_out[:, :], in_=lgT[:, :])

    nc.finalize()
    return nc


def _col_layout(vec, nch):
    return np.ascontiguousarray(np.asarray(vec, np.float32).reshape(nch, P).T)


def _rep(vec):
    v = np.asarray(vec).astype(BF)
    return np.ascontiguousarray(np.broadcast_to(v[None, :], (P, v.shape[0])))


def prepare_inputs(inputs):
    f = {k: np.asarray(v) for k, v in inputs.items()}
    # fold LN gains into the following linear layers (all fp32 on host):
    # fc' = diag(norm_g) @ fc_w, fc_b' = fc_b + norm_b @ fc_w
    fc_w_f = np.asarray(f["norm_g"], np.float32)[:, None] * np.asarray(f["fc_w"], np.float32)
    fc_b_f = np.asarray(f["fc_b"], np.float32) + np.asarray(f["norm_b"], np.float32) @ np.asarray(f["fc_w"], np.float32)
    f1_w_f = np.asarray(f["ln2_g"], np.float32)[:, :, None] * np.asarray(f["f1_w"], np.float32)
    f1_b_f = np.asarray(f["f1_b"], np.float32) + np.einsum(
        "ld,ldo->lo", np.asarray(f["ln2_b"], np.float32), np.asarray(f["f1_w"], np.float32))
    lin_w_f = np.asarray(f["lnf_g"], np.float32)[:, None] * np.asarray(f["lin_w"], np.float32)
    lin_b_f = np.asarray(f["lin_b"], np.float32) + np.asarray(f["lnf_b"], np.float32) @ np.asarray(f["lin_w"], np.float32)
    wmap = {
        "te_w": f["tok_embed"].astype(BF),
        "pos_e": f["pos_embed"][0].astype(BF),
        "fc_w": fc_w_f.astype(BF),
        "wq_w": f["wq_w"].astype(BF),
        "wk_w": f["wk_w"].astype(BF),
        "wv_w": f["wv_w"].astype(BF),
        "ao_w": f["ao_w"].astype(BF),
        "f1_w": f1_w_f.astype(BF),
        "f2_w": f["f2_w"].astype(BF),
        "lin_w": lin_w_f.astype(BF),
        "head_w": f["head_w"].astype(BF),
                "fc_b": _rep(fc_b_f),
        "lin_b": _col_layout(lin_b_f, CD),
        "ln1_g": np.stack([_rep(f["ln1_g"][l]) for l in range(L)]),
        "ln1_b": np.stack([_rep(f["ln1_b"][l]) for l in range(L)]),
        "ao_b": np.stack([_rep(f["ao_b"][l]) for l in range(L)]),
        "f2_b": np.stack([_rep(f["f2_b"][l]) for l in range(L)]),
        "wq_b": np.stack([_col_layout(f["wq_b"][l], CD) for l in range(L)]),
        "wk_b": np.stack([_col_layout(f["wk_b"][l], CD) for l in range(L)]),
        "wv_b": np.stack([_col_layout(f["wv_b"][l], CD) for l in range(L)]),
        "f1_b": np.stack([_col_layout(f1_b_f[l], CF) for l in range(L)]),
    }
    seqs = np.asarray(f["masked_seqs"]).astype(np.int64)
    poss = np.asarray(f["masked_pos"]).astype(np.int64)
    in_maps = []
    for b in range(B):
        seq_oh = (seqs[b][None, :] == np.arange(V)[:, None]).astype(BF)
        pos_oh = (poss[b][None, :] == np.arange(T)[:, None]).astype(BF)
        maskrow = np.where(seqs[b] == 0, 0.0, 1.0).astype(np.float32)
        maskc = np.ascontiguousarray(maskrow.reshape(TQ, P).T)  # [k%128, k//128]
        in_maps.append({"seq_oh": seq_oh, "pos_oh": np.ascontiguousarray(pos_oh),
                        "maskc": maskc, "maskcb": maskc.astype(BF), "rsel": _RSEL, **wmap})
    return in_maps


_RSEL = np.zeros((1, 2, P), BF)
_RSEL[0, 0, 0:64] = 1
_RSEL[0, 1, 64:128] = 1

_GRAPH_CACHE = {}


def run(inputs, trace=False):
    in_maps = prepare_inputs(inputs)
    if "nc" not in _GRAPH_CACHE:
        _GRAPH_CACHE["nc"] = build_graph()
    nc = _GRAPH_CACHE["nc"]
    res = run_bass_kernel_spmd(nc, in_maps, core_ids=list(range(B)), trace=trace)
    logits = np.stack([np.ascontiguousarray(np.asarray(r["logitsT"]).T) for r in res.results])
    lg = logits.astype(np.float32)
    mx = lg.max(-1, keepdims=True)
    logp = lg - mx - np.log(np.exp(lg - mx).sum(-1, keepdims=True))
    tok = np.asarray(inputs["masked_tokens"]).astype(np.int64)
    picked = np.take_along_axis(logp, tok[:, :, None], axis=-1)
    loss = np.float32(-picked.mean())
    return (logits, loss), res


def kernel(**inputs):
    (logits, loss), _ = run(inputs, trace=False)
    return logits, loss
